# revision 6
# baseline (speedup 1.0000x reference)
"""Pointer-network (enc LSTM -> dec LSTM + attention) Trainium2 Bass kernel, v2.

Sharding: pure data-parallel over batch B=256 across 8 NeuronCores (32/core).
Everything SBUF-resident per core; sequential scan over L stays on-core.

v2 vs v1: the v1 kernel issued ~420 matmuls per decode step (weight-stationary
N=32 gates, per-(b,hc) context matvecs) and was PE/LDWEIGHTS-bound. v2:
  * LSTM state kept in replicated batch layout h2/C [(r,b)=128, 512] (4
    replicas of the 32-batch rows), so gates are 36 matmuls with N=512 moving
    (stationary = h columns, moving = weight rows) and the pointwise runs on
    [128, 512] tiles.
  * h stored doubled (h2 = 2h) so sigmoid-from-tanh needs no +1/2 fixups;
    all h-consumer weights are pre-halved on the host.
  * context = sum_l a[b,l] * enc[l,b,h] computed on DVE in the (lb,b)
    partition layout (Hl3 [(lb,b), h, l'] written for free by the encoder's
    replicated h), with the softmax 1/s normalization folded into a tiny
    mask-matmul that also reduces the 4 lb partial groups.
  * softmax has no max-subtraction (scores are bounded); log-softmax is
    deferred: raw scores go to DRAM, one Ln + batched subtract at the end.
Per decode step: ~190 tensor-engine instructions (was ~420), DVE ~30us.

Layouts (p = partition dim):
  h2/C      [128 (r,b), 512]      r = replica; h2 = 2*h
  hT        [128 ksub, 4 kc, 32 b]  transposed h2 (kc*128+ksub = k)
  E_sb/T_sb [128 hsub, 128 l, 128 (hc,b)]
  Hl3       [128 (lb,b), 512 h, 32 lp]   enc h2 at l = lb*32+lp
  gates     [128 (r,b), 2048] PSUM (i|f|g|o)
"""

import os
import sys

import numpy as np

for _p in ("/opt/trn_rl_repo", os.environ.get("TRN_RL_REPO", "")):
    if _p and _p not in sys.path and os.path.isdir(_p):
        sys.path.insert(0, _p)

import ml_dtypes

bf16 = ml_dtypes.bfloat16
fp16 = np.float16

B, L, H = 256, 128, 512
NCORES = 8
BL = B // NCORES  # 32
HC = H // 128     # 4

_cache = {}


def _build_nc(enc_steps=L, dec_steps=L):
    import concourse.bass as bass
    import concourse.bacc as bacc
    import concourse.tile as tile
    from concourse import mybir
    from concourse.masks import make_identity

    AFT = mybir.ActivationFunctionType
    ALU = mybir.AluOpType
    f32 = mybir.dt.float32
    b16 = mybir.dt.bfloat16
    f16 = mybir.dt.float16

    nc = bacc.Bacc("TRN2", target_bir_lowering=False, debug=False)

    xa3_d = nc.dram_tensor("xa3", [3, L, 128], b16, kind="ExternalInput").ap()
    dt3_d = nc.dram_tensor("dt3", [3, L, 128], b16, kind="ExternalInput").ap()
    ewb3_d = nc.dram_tensor("ewb3", [3, 4 * H], b16, kind="ExternalInput").ap()
    dwb3_d = nc.dram_tensor("dwb3", [3, 4 * H], b16, kind="ExternalInput").ap()
    ewhR_d = nc.dram_tensor("ewhR", [128, HC, 4 * H], b16, kind="ExternalInput").ap()
    dwhR_d = nc.dram_tensor("dwhR", [128, HC, 4 * H], b16, kind="ExternalInput").ap()
    dwiR_d = nc.dram_tensor("dwiR", [128, HC, 4 * H], b16, kind="ExternalInput").ap()
    w1T_d = nc.dram_tensor("w1T", [128, HC, H], b16, kind="ExternalInput").ap()
    w2T_d = nc.dram_tensor("w2T", [128, HC, H], b16, kind="ExternalInput").ap()
    v4_d = nc.dram_tensor("v4", [128, HC], b16, kind="ExternalInput").ap()
    outp_d = nc.dram_tensor("outp", [BL, L, L], f32, kind="ExternalOutput").ap()
    sstore_d = nc.dram_tensor("sstore", [L, BL, L], f32).ap()

    with tile.TileContext(nc) as tc, tc.tile_pool(name="perm", bufs=1) as perm:
        E_sb = perm.tile([128, L, 128], b16)
        Hl3 = perm.tile([128, H, 32], b16)
        dwhR = perm.tile([128, HC, 4 * H], b16)
        dwiR = perm.tile([128, HC, 4 * H], b16)
        w1T = perm.tile([128, HC, H], b16)
        w2T = perm.tile([128, HC, H], b16)
        dwb3 = perm.tile([3, 4 * H], b16)
        v4 = perm.tile([128, HC], b16)
        eye = perm.tile([128, 128], f32)
        eye16 = perm.tile([128, 128], b16)
        mask4 = perm.tile([128, BL], b16)     # mask4[r*32+b, b'] = (b==b')
        mask4f = perm.tile([128, BL], f32)
        mask4T = perm.tile([BL, 128], f32)    # transpose of mask4
        h2 = perm.tile([128, H], b16)         # 2*h, replicated layout
        C = perm.tile([128, H], b16)          # 2*c
        hT = perm.tile([128, HC, BL], b16)
        hT4 = perm.tile([128, HC, 4, BL], b16)
        s_all = perm.tile([BL, L], f32)

        nc.sync.dma_start(dwhR, dwhR_d)
        nc.sync.dma_start(dwiR, dwiR_d)
        nc.sync.dma_start(w1T, w1T_d)
        nc.sync.dma_start(w2T, w2T_d)
        nc.sync.dma_start(dwb3, dwb3_d)
        nc.sync.dma_start(v4, v4_d)
        make_identity(nc, eye)
        nc.vector.tensor_copy(eye16, eye)
        # mask4[p, b'] = (p % 32 == b'): build from identity rows
        for r in range(4):
            nc.vector.tensor_copy(mask4[r * 32:(r + 1) * 32, :],
                                  eye16[0:32, 0:32])
            nc.vector.tensor_copy(mask4f[r * 32:(r + 1) * 32, :],
                                  eye[0:32, 0:32])
            nc.vector.tensor_copy(mask4T[:, r * 32:(r + 1) * 32],
                                  eye[0:32, 0:32])
        nc.vector.memset(h2, 0.0)
        nc.vector.memset(C, 0.0)
        nc.vector.memset(hT, 0.0)
        nc.vector.memset(hT4, 0.0)
        nc.vector.memset(s_all, 1.0)
        if enc_steps < L:
            nc.vector.memset(E_sb, 0.0)
            nc.vector.memset(Hl3, 0.0)

        def lstm_pointwise(work, g_ps):
            """g_ps [128, 2048] PSUM -> update h2, C (doubled state).

            sigma(x) = (1+tanh(x/2))/2; with h2=2h, C=2c:
            C_new = 0.5*C + 0.5*C*tf + tg + ti*tg;  tc = tanh(0.5*C_new)
            h2_new = tc + to*tc
            """
            ti = work.tile([128, H], b16, tag="ti")
            tf = work.tile([128, H], b16, tag="tf")
            tg = work.tile([128, H], b16, tag="tg")
            to = work.tile([128, H], b16, tag="to")
            nc.scalar.activation(ti, g_ps[:, 0 * H:1 * H], AFT.Tanh, scale=0.5)
            nc.scalar.activation(tf, g_ps[:, 1 * H:2 * H], AFT.Tanh, scale=0.5)
            nc.scalar.activation(tg, g_ps[:, 2 * H:3 * H], AFT.Tanh, scale=1.0)
            nc.scalar.activation(to, g_ps[:, 3 * H:4 * H], AFT.Tanh, scale=0.5)
            u = work.tile([128, H], b16, tag="u")
            w = work.tile([128, H], b16, tag="w")
            z = work.tile([128, H], b16, tag="z")
            y = work.tile([128, H], b16, tag="y")
            nc.vector.scalar_tensor_tensor(out=u, in0=C, scalar=0.5, in1=tf,
                                           op0=ALU.mult, op1=ALU.mult)
            nc.vector.scalar_tensor_tensor(out=w, in0=C, scalar=0.5, in1=u,
                                           op0=ALU.mult, op1=ALU.add)
            nc.vector.tensor_mul(z, ti, tg)
            nc.vector.tensor_add(y, tg, z)
            nc.vector.tensor_add(C, w, y)
            tc_ = work.tile([128, H], b16, tag="tc")
            nc.scalar.activation(tc_, C, AFT.Tanh, scale=0.5)
            z2 = work.tile([128, H], b16, tag="z2")
            nc.vector.tensor_mul(z2, to, tc_)
            nc.vector.tensor_add(h2, tc_, z2)

        def h_transposes(psp, write_Hl3_at=None):
            """h2 [(r,b), 512] -> hT [ksub, kc, b], hT4; optionally Hl3 col."""
            tr_ps = psp.tile([128, HC, BL], b16, tag="sm")
            for kc in range(HC):
                nc.tensor.transpose(tr_ps[:, kc, :],
                                    h2[0:32, kc * 128:(kc + 1) * 128],
                                    eye16[0:32, 0:32])
            nc.vector.tensor_copy(hT, tr_ps)
            hT4v = bass.AP(tensor=hT.tensor, offset=hT.offset,
                           ap=[hT.ap[0], hT.ap[1], [0, 4], hT.ap[2]])
            nc.vector.tensor_copy(hT4, hT4v)
            if write_Hl3_at is not None:
                lb, lp = write_Hl3_at
                dst = Hl3[lb * 32:(lb + 1) * 32, :, lp:lp + 1]
                nc.vector.tensor_copy(
                    dst, h2[lb * 32:(lb + 1) * 32, :].unsqueeze(-1))

        # ---------------- encoder ----------------
        with tc.tile_pool(name="encp", bufs=1) as encp, \
             tc.tile_pool(name="encw", bufs=1) as encw, \
             tc.tile_pool(name="encr", bufs=4) as encr, \
             tc.tile_pool(name="psg", bufs=1, space="PSUM") as psg, \
             tc.tile_pool(name="pse", bufs=2, space="PSUM") as pse, \
             tc.tile_pool(name="pst", bufs=2, space="PSUM") as pst:
            ewhR = encp.tile([128, HC, 4 * H], b16)
            ewb3 = encp.tile([3, 4 * H], b16)
            nc.sync.dma_start(ewhR, ewhR_d)
            nc.sync.dma_start(ewb3, ewb3_d)

            for t in range(enc_steps):
                xa_t = encr.tile([3, 128], b16, tag="xa")
                nc.sync.dma_start(xa_t, xa3_d[:, t, :])
                g_ps = psg.tile([128, 4 * H], mybir.dt.float32, tag="gps")
                for bank in range(4):
                    sl = slice(bank * H, (bank + 1) * H)
                    for kc in range(HC):
                        nc.tensor.matmul(
                            g_ps[:, sl], lhsT=hT4[:, kc, :, :],
                            rhs=ewhR[:, kc, sl],
                            start=(kc == 0), stop=False)
                    nc.tensor.matmul(g_ps[:, sl], lhsT=xa_t,
                                     rhs=ewb3[:, sl], start=False, stop=True)
                lstm_pointwise(encw, g_ps)
                h_transposes(pst, write_Hl3_at=(t // 32, t % 32))
                # E row: 16 weight-stationary matmuls -> [hsub, hc', b]
                e_ps = pse.tile([128, HC, BL], mybir.dt.float32, tag="eps")
                for pc in range(HC):
                    for kc in range(HC):
                        nc.tensor.matmul(
                            e_ps[:, pc, :],
                            lhsT=w1T[:, kc, pc * 128:(pc + 1) * 128],
                            rhs=hT[:, kc, :], start=(kc == 0), stop=(kc == 3))
                nc.vector.tensor_copy(
                    E_sb[:, t:t + 1, :],
                    e_ps.rearrange("p a b -> p (a b)").unsqueeze(1))

        # ---------------- decoder ----------------
        with tc.tile_pool(name="decp", bufs=1) as decp, \
             tc.tile_pool(name="decw", bufs=1) as decw, \
             tc.tile_pool(name="decx", bufs=2) as decx, \
             tc.tile_pool(name="decr", bufs=4) as decr, \
             tc.tile_pool(name="dece", bufs=1) as dece, \
             tc.tile_pool(name="psq", bufs=1, space="PSUM") as psq, \
             tc.tile_pool(name="psz", bufs=1, space="PSUM") as psz, \
             tc.tile_pool(name="psg2", bufs=1, space="PSUM") as psg2, \
             tc.tile_pool(name="pssm", bufs=2, space="PSUM") as pssm:
            T_sb = decp.tile([128, L, 128], b16)

            for t in range(dec_steps):
                # q: 16 weight-stationary matmuls -> q_ps [hsub, hc', b]
                q_ps = psq.tile([128, HC, BL], mybir.dt.float32, tag="qps")
                for pc in range(HC):
                    for kc in range(HC):
                        nc.tensor.matmul(
                            q_ps[:, pc, :],
                            lhsT=w2T[:, kc, pc * 128:(pc + 1) * 128],
                            rhs=hT[:, kc, :], start=(kc == 0), stop=(kc == 3))
                # gates, part 1: Wh*h into 4 PSUM banks (overlaps attention)
                g_ps = psg2.tile([128, 4 * H], mybir.dt.float32, tag="gps2")
                for bank in range(4):
                    sl = slice(bank * H, (bank + 1) * H)
                    for kc in range(HC):
                        nc.tensor.matmul(
                            g_ps[:, sl], lhsT=hT4[:, kc, :, :],
                            rhs=dwhR[:, kc, sl],
                            start=(kc == 0), stop=False)
                qT = decw.tile([128, HC, BL], b16, tag="qT")
                nc.vector.tensor_copy(qT, q_ps)
                qflat = qT.rearrange("p a b -> p (a b)")
                # X = E + q (broadcast over l), T = tanh(X), scores per l-block
                Z_ps = psz.tile([128, L, HC], mybir.dt.float32, tag="zps")
                for blk in range(4):
                    lsl = slice(blk * 32, (blk + 1) * 32)
                    X_blk = decx.tile([128, 32, 128], b16, tag="X")
                    q_b = bass.AP(tensor=qflat.tensor, offset=qflat.offset,
                                  ap=[qflat.ap[0], [0, 32], qflat.ap[1]])
                    nc.vector.tensor_add(X_blk, E_sb[:, lsl, :], q_b)
                    nc.scalar.activation(T_sb[:, lsl, :], X_blk,
                                         AFT.Tanh, scale=1.0)
                    for l in range(blk * 32, (blk + 1) * 32):
                        nc.tensor.matmul(Z_ps[:, l, :], lhsT=T_sb[:, l, :],
                                         rhs=v4, start=True, stop=True)
                # S[b, l] = sum_hc Z[(hc,b), l, hc]
                S_sb = decw.tile([BL, L], mybir.dt.float32, tag="S")
                nc.vector.tensor_copy(S_sb, Z_ps[0:32, :, 0])
                nc.vector.tensor_add(S_sb, S_sb, Z_ps[32:64, :, 1])
                nc.vector.tensor_add(S_sb, S_sb, Z_ps[64:96, :, 2])
                nc.vector.tensor_add(S_sb, S_sb, Z_ps[96:128, :, 3])
                nc.sync.dma_start(sstore_d[t, :, :], S_sb)
                # softmax pieces (no max subtraction; scores are bounded)
                e_sb = decw.tile([BL, L], mybir.dt.float32, tag="e")
                nc.scalar.activation(e_sb, S_sb, AFT.Exp, scale=1.0)
                nc.vector.tensor_reduce(out=s_all[:, t:t + 1], in_=e_sb,
                                        axis=mybir.AxisListType.X, op=ALU.add)
                r_sb = decw.tile([BL, 1], mybir.dt.float32, tag="r")
                nc.vector.reciprocal(r_sb, s_all[:, t:t + 1])
                # e3 [(lb,b), lp]: partition regroup of e_sb via 4 local DMAs
                e3 = decw.tile([128, 32], b16, tag="e3")
                e_sb16 = decw.tile([BL, L], b16, tag="e16")
                nc.vector.tensor_copy(e_sb16, e_sb)
                for lb in range(4):
                    nc.sync.dma_start(e3[lb * 32:(lb + 1) * 32, :],
                                      e_sb16[:, lb * 32:(lb + 1) * 32])
                # r3 [(lb,b), 1] = r[b] replicated; maskR = mask4 * r3
                r3_ps = pssm.tile([128, 1], mybir.dt.float32, tag="sm")
                nc.tensor.matmul(r3_ps, lhsT=mask4T, rhs=r_sb,
                                 start=True, stop=True)
                r3 = decw.tile([128, 1], mybir.dt.float32, tag="r3s")
                nc.vector.tensor_copy(r3, r3_ps)
                maskR = decw.tile([128, BL], b16, tag="mR")
                nc.vector.tensor_scalar_mul(maskR, mask4, r3)
                # context partials: AH = Hl3 * e3 (bcast over h), reduce lp
                ctx3 = decw.tile([128, H], mybir.dt.float32, tag="ctx3")
                for ch in range(2):
                    hsl = slice(ch * 256, (ch + 1) * 256)
                    AH = dece.tile([128, 256, 32], f16, tag="AH")
                    e3b = bass.AP(tensor=e3.tensor, offset=e3.offset,
                                  ap=[e3.ap[0], [0, 256], e3.ap[1]])
                    nc.vector.tensor_mul(AH, Hl3[:, hsl, :], e3b)
                    nc.vector.tensor_reduce(out=ctx3[:, hsl], in_=AH,
                                            axis=mybir.AxisListType.X,
                                            op=ALU.add)
                ctx3c = decw.tile([128, H], b16, tag="ctx3c")
                nc.vector.tensor_copy(ctx3c, ctx3)
                # fold lb groups + 1/s scale: ctxT[ksub, kc, b]
                ctxT_ps = pssm.tile([128, HC, BL], mybir.dt.float32, tag="sm")
                for kc in range(HC):
                    nc.tensor.matmul(ctxT_ps[:, kc, :],
                                     lhsT=ctx3c[:, kc * 128:(kc + 1) * 128],
                                     rhs=maskR, start=True, stop=True)
                ctxT4 = decw.tile([128, HC, 4, BL], b16, tag="ctxT4")
                cT4v = bass.AP(tensor=ctxT_ps.tensor, offset=ctxT_ps.offset,
                               ap=[ctxT_ps.ap[0], ctxT_ps.ap[1], [0, 4],
                                   ctxT_ps.ap[2]])
                nc.vector.tensor_copy(ctxT4, cT4v)
                # gates, part 2: Wi*ctx + Wi_d*d + bias
                dt_t = decr.tile([3, 128], b16, tag="dt")
                nc.sync.dma_start(dt_t, dt3_d[:, t, :])
                for bank in range(4):
                    sl = slice(bank * H, (bank + 1) * H)
                    for kc in range(HC):
                        nc.tensor.matmul(
                            g_ps[:, sl], lhsT=ctxT4[:, kc, :, :],
                            rhs=dwiR[:, kc, sl], start=False, stop=False)
                    nc.tensor.matmul(g_ps[:, sl], lhsT=dt_t,
                                     rhs=dwb3[:, sl], start=False, stop=True)
                lstm_pointwise(decw, g_ps)
                if t + 1 < dec_steps:
                    h_transposes(pssm)

        # ---------------- deferred log-softmax ----------------
        with tc.tile_pool(name="post", bufs=4) as post, \
             tc.tile_pool(name="postc", bufs=1) as postc:
            lnm = postc.tile([BL, L], mybir.dt.float32)
            nc.scalar.activation(lnm, s_all, AFT.Ln, scale=1.0)
            TB = 8
            for t0 in range(0, dec_steps, TB):
                S_t = post.tile([BL, TB, L], mybir.dt.float32, tag="St")
                nc.sync.dma_start(S_t, sstore_d[t0:t0 + TB, :, :].rearrange(
                    "t b l -> b t l"))
                o_t = post.tile([BL, TB, L], mybir.dt.float32, tag="ot")
                lsl = lnm[:, t0:t0 + TB]
                lnb = bass.AP(tensor=lsl.tensor, offset=lsl.offset,
                              ap=[lsl.ap[0], lsl.ap[1], [0, L]])
                nc.vector.tensor_sub(o_t, S_t, lnb)
                nc.sync.dma_start(outp_d[:, t0:t0 + TB, :], o_t)

    nc.finalize()
    return nc


def _hi_lo(x):
    hi = x.astype(bf16).astype(np.float32)
    lo = (x - hi).astype(bf16)
    return hi.astype(bf16), lo


def _prep_weights(enc_Wi, enc_Wh, enc_b, dec_Wi, dec_Wh, dec_b, w1, w2, vt):
    """Host-side weight repack (shared across cores). h-consumers halved
    because the kernel stores h2 = 2h."""
    f = np.float32

    def rows(W, scale):  # [4H/H, H] -> [128, HC, out]: r[p, kc, g] = s*W[g, kc*128+p]
        Wt = np.ascontiguousarray(W.astype(f).T) * scale  # [H, out]
        return Wt.reshape(HC, 128, W.shape[0]).transpose(1, 0, 2).astype(bf16)

    ewb3 = np.stack([enc_Wi.astype(f)[:, 0], enc_Wi.astype(f)[:, 0],
                     enc_b.astype(f)]).astype(bf16)
    # decoder input col hi/lo handled in dt3; bias row here
    dwb3 = np.stack([dec_Wi.astype(f)[:, H], dec_Wi.astype(f)[:, H],
                     dec_b.astype(f)]).astype(bf16)
    return {
        "ewb3": ewb3, "dwb3": dwb3,
        "ewhR": rows(enc_Wh, 0.5), "dwhR": rows(dec_Wh, 0.5),
        "dwiR": rows(dec_Wi[:, :H], 0.5),
        "w1T": rows(w1, 0.5), "w2T": rows(w2, 0.5),
        "v4": vt.astype(f)[0].reshape(HC, 128).T.astype(bf16).copy(),
    }


def kernel(xs, x_lens, argsort_xs, enc_Wi, enc_Wh, enc_b,
           dec_Wi, dec_Wh, dec_b, w1, w2, vt):
    from concourse.bass_utils import run_bass_kernel_spmd

    if "nc" not in _cache:
        _cache["nc"] = _build_nc()
    nc = _cache["nc"]

    wmap = _prep_weights(enc_Wi, enc_Wh, enc_b, dec_Wi, dec_Wh, dec_b,
                         w1, w2, vt)
    xs_f = xs.astype(np.float32)
    D = np.concatenate(
        [np.zeros((B, 1), np.float32),
         np.take_along_axis(xs_f, argsort_xs[:, :-1].astype(np.int64), axis=1)],
        axis=1)  # [B, L] teacher-forced decoder inputs

    in_maps = []
    for c in range(NCORES):
        sl = slice(c * BL, (c + 1) * BL)
        # xa3 [3, L, 128]: rows xhi, xlo, ones; cols (r,b) replicated
        xhi, xlo = _hi_lo(xs_f[sl].T)            # [L, BL]
        ones = np.ones((L, BL), bf16)
        xa3 = np.stack([xhi, xlo, ones]).astype(bf16)      # [3, L, BL]
        xa3 = np.tile(xa3, (1, 1, 4))                      # [3, L, 128]
        dhi, dlo = _hi_lo(D[sl].T)
        dt3 = np.stack([dhi, dlo, ones]).astype(bf16)
        dt3 = np.tile(dt3, (1, 1, 4))
        m = dict(wmap)
        m["xa3"] = xa3
        m["dt3"] = dt3
        in_maps.append(m)

    _cache["in_maps"] = in_maps
    res = run_bass_kernel_spmd(nc, in_maps, core_ids=list(range(NCORES)))
    out = np.concatenate([res.results[c]["outp"] for c in range(NCORES)], axis=0)
    return np.ascontiguousarray(out.astype(np.float32))
